# revision 26
# baseline (speedup 1.0000x reference)
"""Trainium2 Bass kernel for nn_NeuralLongTermMemory (B=4, S=4096, D=1024).

Data-parallel over the 16384 tokens across 8 NeuronCores (2048 tokens/core):
forward projections + inner-loop analytic gradients locally; each
[1024,1024] gradient outer-product partial (they already sum over tokens)
is bf16-ReduceScattered, the owning core applies clip/lr/decay to its
128-row shard, and the updated fast weight is bf16-AllGathered.  Scalar
gates come from an early tiny AllReduce of per-core x column-sums.  The
retrieval tail fuses W1n and Wout into one weight (no nonlinearity between
them): out = z @ (Wout @ W1n)^T, where the base term (Wout@W1)^T is
host-precomputed and the gradient term (-lr clip(g1))^T @ Wout^T is a tiny
per-shard matmul folded into the second AllGather — this removes one full
[16384x1024x1024] matmul from the post-collective critical path.

Layouts: activations feature-major ([d partitions, t free]) so every Linear
is lhsT=W^T (host-pretransposed bf16), rhs=activation.  The gradient
matmuls contract over tokens; of their four token-major operands, k and d1
are produced token-major directly (k via the stationary-operand swap on x —
enabled by additionally fusing Wk into W0 for pre1 = x @ (W0@Wk)^T, which
removes k from the forward dependency chain; d1 as (d2 @ W1)_tok multiplied
by a DRAM-streamed transposed silu' — so neither needs a transpose round
trip), while a1 and d2 are re-materialized via DRAM-bounce DMA-transposes,
all timed to finish before the first gradient collective (Tile serializes
DMA-transposes against in-flight collectives, and the xbar paces them at
~3 us each, so transpose count is the scarce resource).  All matmuls bf16
with fp32 PSUM accumulation.

SBUF plan: four 4MB activation buffers (bA..bD) with manually scheduled
role reuse; q and silu'(pre1) are spilled to DRAM (write-once in phase A,
streamed back later) to keep the peak working set at four tensors.
"""

import numpy as np
import ml_dtypes

import concourse.bacc as bacc
import concourse.mybir as mybir
import concourse.tile as tile
from concourse.bass_utils import run_bass_kernel_spmd

BF16 = mybir.dt.bfloat16
F32 = mybir.dt.float32

B, S, D = 4, 4096, 1024
NC = 8
T = B * S // NC          # 2048 tokens per core
P = 128
EB = D // P              # 8 feature blocks
TBL = T // P             # 16 token blocks
NTC = T // 512           # 4 token chunks of 512
ERR_CLIP = 5.0
GRAD_CLIP = 1.0
D2_SCALE = 2.0 / float(S)
MEAN_N = float(B * S)

_CACHE = {}
NO_CC = False


def _build(reps=1):
    nc = bacc.Bacc(None, target_bir_lowering=False, num_devices=NC)

    # ---------------- DRAM I/O ----------------
    xT_d = nc.dram_tensor("xT", [D, T], BF16, kind="ExternalInput")
    wkt_d = nc.dram_tensor("WKT", [D, D], BF16, kind="ExternalInput")
    wvt_d = nc.dram_tensor("WVT", [D, D], BF16, kind="ExternalInput")
    wqt_d = nc.dram_tensor("WQT", [D, D], BF16, kind="ExternalInput")
    w0k_d = nc.dram_tensor("W0KT", [D, D], BF16, kind="ExternalInput")
    w1t_d = nc.dram_tensor("W1T", [D, D], BF16, kind="ExternalInput")
    w1n_d = nc.dram_tensor("W1N", [D, D], BF16, kind="ExternalInput")  # W1 as stored
    wot_d = nc.dram_tensor("WOT", [D, D], BF16, kind="ExternalInput")
    w0s_d = nc.dram_tensor("W0TS", [P, D], BF16, kind="ExternalInput")
    wfs_d = nc.dram_tensor("WFS", [P, D], BF16, kind="ExternalInput")
    wg_d = nc.dram_tensor("WG", [P, 2, EB], F32, kind="ExternalInput")
    bg_d = nc.dram_tensor("BG", [1, 2], F32, kind="ExternalInput")
    out_d = nc.dram_tensor("out", [T, D], F32, kind="ExternalOutput")

    def wt_view(d):  # [D, D] -> [p, eb, n] SBUF-layout view
        return d.rearrange("(eb p) n -> p eb n", p=P)

    with tile.TileContext(nc) as tc:
        with (
            tc.tile_pool(name="act", bufs=1) as act,
            tc.tile_pool(name="wt", bufs=2) as wt,
            tc.tile_pool(name="ps", bufs=7, space="PSUM") as ps,
            tc.tile_pool(name="psg", bufs=1, space="PSUM") as psg,
            tc.tile_pool(name="stage", bufs=3) as stage,
            tc.tile_pool(name="qcp", bufs=2) as qcp,
            tc.tile_pool(name="outst", bufs=2) as outst,
            tc.tile_pool(name="gup", bufs=1) as gup,
            tc.tile_pool(name="gate", bufs=1) as gate,
            tc.tile_pool(name="dram", bufs=1, space="DRAM") as dram,
        ):
          for _rep in range(reps):
            AS = [P, EB, T]   # feature-major activation [p, d-block, t]
            TS = [P, TBL, D]  # token-major activation [p, t-block, d]
            WS = [P, EB, D]   # weight [p, e-block, n]

            def new_act(tag, name):
                return act.tile(AS, BF16, tag=tag, name=name)

            def new_tok(tag, name):
                return act.tile(TS, BF16, tag=tag, name=name)

            def new_wt(dram_t, name):
                w = wt.tile(WS, BF16, tag="wt", name=name)
                wv_ap = wt_view(dram_t)
                for eb in range(EB):
                    nc.sync.dma_start(w[:, eb, :], wv_ap[:, eb, :])
                return w

            def linear(w_sb, in_sb, post):
                """feature-major psum[ob*128+p, t] = sum_e W^T[e, ob] in[e, t].

                post(psum_ap, ob, tci) finalizes each [128, 512] block.
                """
                for ob in range(EB):
                    pts = [
                        ps.tile([P, 512], F32, tag="mm", name=f"mm_{ob}_{i}")
                        for i in range(NTC)
                    ]
                    for e in range(EB):
                        for tci in range(NTC):
                            nc.tensor.matmul(
                                pts[tci][:],
                                w_sb[:, e, ob * P : (ob + 1) * P],
                                in_sb[:, e, tci * 512 : (tci + 1) * 512],
                                start=(e == 0),
                                stop=(e == EB - 1),
                            )
                    for tci in range(NTC):
                        post(pts[tci][:], ob, tci)

            def copy_into(dst):
                def _p(pt, ob, tci):
                    nc.vector.tensor_copy(
                        out=dst[:, ob, tci * 512 : (tci + 1) * 512], in_=pt
                    )
                return _p

            def silu_into(dst):
                def _p(pt, ob, tci):
                    nc.scalar.activation(
                        out=dst[:, ob, tci * 512 : (tci + 1) * 512], in_=pt,
                        func=mybir.ActivationFunctionType.Silu,
                    )
                return _p

            # ------- Phase A: x -> k_tok, v, q(spill), pre1 -> a1, sd(spill) --
            # Wk is fused into W0 for pre1 = x @ (W0 @ Wk)^T (no nonlinearity
            # between the k and pre1 Linears); k itself is produced directly
            # token-major (stationary-operand swap), so it needs no transpose.
            xT = new_act("bA", "xT")
            xview = xT_d.rearrange("(eb p) t -> p eb t", p=P)
            for eb in range(EB):
                nc.sync.dma_start(xT[:, eb, :], xview[:, eb, :])

            wk = new_wt(wkt_d, "WKT")
            wv = new_wt(wvt_d, "WVT")

            k_tok = new_tok("bB", "k_tok")
            for tb in range(TBL):
                for dc in range(2):
                    pt = ps.tile([P, 512], F32, tag="mm", name=f"kmm{tb}_{dc}")
                    for e in range(EB):
                        nc.tensor.matmul(
                            pt[:],
                            xT[:, e, tb * P : (tb + 1) * P],
                            wk[:, e, dc * 512 : (dc + 1) * 512],
                            start=(e == 0),
                            stop=(e == EB - 1),
                        )
                    nc.vector.tensor_copy(
                        out=k_tok[:, tb, dc * 512 : (dc + 1) * 512], in_=pt[:]
                    )

            vT = new_act("bC", "vT")
            linear(wv, xT, copy_into(vT))

            # q is spilled: psum -> bf16 stage -> DRAM bounce (streamed back
            # chunk-wise for retrieval, long after the gradient phase)
            bq = dram.tile([D, T], BF16, tag="bq", name="bq")

            def q_post(pt, ob, tci):
                st = stage.tile([P, 512], BF16, tag="qst", name="qst")
                nc.vector.tensor_copy(out=st[:], in_=pt)
                nc.sync.dma_start(
                    bq[ob * P : (ob + 1) * P, tci * 512 : (tci + 1) * 512], st[:]
                )

            wq = new_wt(wqt_d, "WQT")
            linear(wq, xT, q_post)

            # pre1 via the fused weight; a1 stays resident, sd spills to DRAM
            w0k = new_wt(w0k_d, "W0KT")
            a1T = new_act("bD", "a1T")
            bsd = dram.tile([D, T], BF16, tag="bsd", name="bsd")

            def b1_post(pt, ob, tci):
                sl = slice(tci * 512, (tci + 1) * 512)
                nc.scalar.activation(
                    out=a1T[:, ob, sl], in_=pt,
                    func=mybir.ActivationFunctionType.Silu,
                )
                sds = stage.tile([P, 512], BF16, tag="sdst", name="sdst")
                nc.scalar.activation(
                    out=sds[:], in_=pt,
                    func=mybir.ActivationFunctionType.Derivative_silu,
                )
                nc.sync.dma_start(
                    bsd[ob * P : (ob + 1) * P, tci * 512 : (tci + 1) * 512],
                    sds[:],
                )

            linear(w0k, xT, b1_post)

            # xsum[p, eb] = sum_t x^T[p, eb, t]  (gate statistics)
            xsum = gate.tile([P, EB], F32)
            nc.vector.reduce_sum(xsum[:, :, None], xT[:], axis=mybir.AxisListType.X)

            # early tiny all-reduce for the gates
            cc_s_in = dram.tile([P, EB], F32, name="cc_s_in")
            cc_s_out = dram.tile([P, EB], F32, name="cc_s_out")
            nc.sync.dma_start(cc_s_in[:], xsum[:])
            if NO_CC:
                nc.sync.dma_start(cc_s_out[:], cc_s_in[:])
            else:
                nc.gpsimd.collective_compute(
                    "AllReduce",
                    mybir.AluOpType.add,
                    replica_groups=[list(range(NC))],
                    ins=[cc_s_in[:].opt()],
                    outs=[cc_s_out[:].opt()],
                )

            def store_bounce(src, name):
                bounce = dram.tile([D, T], BF16, tag="tb_" + name, name="tb_" + name)
                nc.sync.dma_start(
                    bounce.rearrange("(eb p) t -> p eb t", p=P), src[:]
                )
                return bounce

            # ---------------- Phase B2: pred -> d2 (into vT) -----------------
            w1 = new_wt(w1t_d, "W1T")
            d2T = vT  # renamed below: d2 overwrites v in place

            def b2_post(pt, ob, tci):
                sl = slice(tci * 512, (tci + 1) * 512)
                blk = d2T[:, ob, sl]
                nc.vector.tensor_tensor(blk, pt, blk, mybir.AluOpType.subtract)
                nc.vector.tensor_scalar(
                    out=blk, in0=blk,
                    scalar1=ERR_CLIP, scalar2=-ERR_CLIP,
                    op0=mybir.AluOpType.min, op1=mybir.AluOpType.max,
                )
                nc.vector.tensor_scalar_mul(blk, blk, D2_SCALE)

            linear(w1, a1T, b2_post)
            ba = store_bounce(a1T, "a1")
            bd2 = store_bounce(d2T, "d2")

            def load_tok(bounce, tag, name):
                tok = new_tok(tag, name)
                for tb in range(TBL):
                    nc.sync.dma_start(
                        out=tok[:, tb, :],
                        in_=bounce[:, tb * P : (tb + 1) * P],
                        transpose=True,
                    )
                return tok

            a1_tok = load_tok(ba, "bD", "a1_tok")  # a1T dead after B2 + store

            # -------- Phase C: d1_tok = (d2 @ W1)_tok * sd_tok (streamed) ----
            w1n = new_wt(w1n_d, "W1N")
            d1_tok = new_tok("bA", "d1_tok")  # xT dead after phase A
            for tb in range(TBL):
                # one [1024,128]->[128,1024] sd_tok transpose per token block
                sdb = stage.tile([P, D], BF16, tag="sdld", name="sdld")
                nc.sync.dma_start(
                    out=sdb[:],
                    in_=bsd[:, tb * P : (tb + 1) * P],
                    transpose=True,
                )
                for ec in range(2):
                    pt = ps.tile([P, 512], F32, tag="mm", name=f"cmm{tb}_{ec}")
                    for db in range(EB):
                        nc.tensor.matmul(
                            pt[:],
                            d2T[:, db, tb * P : (tb + 1) * P],
                            w1n[:, db, ec * 512 : (ec + 1) * 512],
                            start=(db == 0),
                            stop=(db == EB - 1),
                        )
                    nc.vector.tensor_tensor(
                        d1_tok[:, tb, ec * 512 : (ec + 1) * 512],
                        pt[:], sdb[:, ec * 512 : (ec + 1) * 512],
                        mybir.AluOpType.mult,
                    )

            d2_tok = load_tok(bd2, "bC", "d2_tok")  # d2T dead after C + store

            # ---------------- Phase E: gradient partials ---------------------
            cc_g0_in = dram.tile([EB, P, D], BF16, name="cc_g0_in")
            rs0_out = dram.tile([P, D], BF16, name="rs0_out")
            ag0_in = dram.tile([P, D], BF16, name="ag0_in")
            ag0_out = dram.tile([EB, P, D], BF16, addr_space="Shared",
                                name="ag0_out")
            cc_g1_in = dram.tile([EB, P, D], BF16, name="cc_g1_in")
            rs1_out = dram.tile([P, D], BF16, name="rs1_out")
            ag1_in = dram.tile([P, D], BF16, name="ag1_in")
            ag1_out = dram.tile([EB, P, D], BF16, addr_space="Shared",
                                name="ag1_out")

            def grad(cc_in, ltok, rtok, mid=None):
                # g^T[eb*128+p, n] = sum_t ltok[t, eb-blk][p] rtok[t, n]
                for eb in range(EB):
                    if eb == EB // 2 and mid is not None:
                        mid()
                    for dc in range(2):
                        pt = ps.tile([P, 512], F32, tag="mm", name=f"gmm{eb}_{dc}")
                        for tb in range(TBL):
                            nc.tensor.matmul(
                                pt[:],
                                ltok[:, tb, eb * P : (eb + 1) * P],
                                rtok[:, tb, dc * 512 : (dc + 1) * 512],
                                start=(tb == 0),
                                stop=(tb == TBL - 1),
                            )
                        st = stage.tile([P, 512], BF16, tag="gst", name="gst")
                        nc.vector.tensor_copy(out=st[:], in_=pt[:])
                        nc.sync.dma_start(
                            cc_in[eb, :, dc * 512 : (dc + 1) * 512], st[:]
                        )

            def reduce_scatter(cc_in, rs_out):
                if NO_CC:
                    return nc.sync.dma_start(rs_out[:], cc_in[0])
                return nc.gpsimd.collective_compute(
                    "ReduceScatter",
                    mybir.AluOpType.add,
                    replica_groups=[list(range(NC))],
                    ins=[cc_in[:].opt()],
                    outs=[rs_out[:].opt()],
                )

            def all_gather(ag_in, ag_out):
                if NO_CC:
                    return nc.gpsimd.dma_start(
                        out=ag_out[:], in_=ag_in[:].partition_broadcast(EB)
                    )
                return nc.gpsimd.collective_compute(
                    "AllGather",
                    mybir.AluOpType.bypass,
                    replica_groups=[list(range(NC))],
                    ins=[ag_in[:].opt()],
                    outs=[ag_out[:].opt()],
                )

            grad(cc_g0_in, k_tok, d1_tok)
            reduce_scatter(cc_g0_in, rs0_out)
            grad(cc_g1_in, a1_tok, d2_tok)
            reduce_scatter(cc_g1_in, rs1_out)
            wo = new_wt(wot_d, "WOT")

            # identity for PE-transpose of the g1 shard
            ident = gate.tile([P, P], BF16)
            from concourse.masks import make_identity
            make_identity(nc, ident[:])

            # ---------------- Gates: 1-alpha, -lr ---------------------------
            wg_sb = gate.tile([P, 2, EB], F32)
            nc.sync.dma_start(wg_sb[:], wg_d[:])
            bg_sb = gate.tile([1, 2], F32)
            nc.sync.dma_start(bg_sb[:], bg_d[:])
            bgneg = gate.tile([1, 2], F32)
            nc.vector.tensor_scalar_mul(bgneg[:], bg_sb[:], -1.0)

            xsg = gate.tile([P, EB], F32)
            nc.sync.dma_start(xsg[:], cc_s_out[:])
            prod = gate.tile([P, 2, EB], F32)
            nc.vector.tensor_tensor(
                prod[:],
                wg_sb[:],
                xsg[:, None, :].to_broadcast((P, 2, EB)),
                mybir.AluOpType.mult,
            )
            rsum = gate.tile([P, 2], F32)
            nc.vector.reduce_sum(rsum[:, :, None], prod[:], axis=mybir.AxisListType.X)
            ones = gate.tile([P, 1], F32)
            nc.vector.memset(ones[:], 1.0)
            pg = psg.tile([1, 2], F32, name="pg")
            nc.tensor.matmul(pg[:], ones[:], rsum[:], start=True, stop=True)

            # sc[0] = 1-alpha = sigmoid(-(s0/N + bg0)); sc[1] = lr
            sc = gate.tile([1, 2], F32)
            nc.scalar.activation(
                out=sc[:, 0:1], in_=pg[:, 0:1],
                func=mybir.ActivationFunctionType.Sigmoid,
                bias=bgneg[:, 0:1], scale=-1.0 / MEAN_N,
            )
            nc.scalar.activation(
                out=sc[:, 1:2], in_=pg[:, 1:2],
                func=mybir.ActivationFunctionType.Sigmoid,
                bias=bg_sb[:, 1:2], scale=1.0 / MEAN_N,
            )
            # sc[1] -> -lr
            nc.vector.tensor_scalar_mul(sc[:, 1:2], sc[:, 1:2], -1.0)
            scb = dram.tile([1, 2], F32, name="scb")
            nc.sync.dma_start(scb[:], sc[:])
            sc_sb = gate.tile([P, 2], F32)
            nc.gpsimd.dma_start(out=sc_sb[:], in_=scb[0].partition_broadcast(P))

            # ---------------- Phase G/H: sharded update + retrieval ---------
            def emit_fused_tail():
                # ---- fused tail: Wf = Wout @ W1n, out = z @ Wf^T ----
                # F^T-shard = (-lr clip(g1))^T-shard @ Wout^T via an [128,1024]
                # PE-transpose + 16 matmuls; combined with the host-precomputed
                # (Wout @ W1)^T base shard, then AllGathered.
                m1sh = gup.tile([P, D], BF16, tag="gblk", name="m1sh")
                nc.sync.dma_start(m1sh[:], rs1_out[:])
                nc.vector.tensor_scalar(
                    out=m1sh[:], in0=m1sh[:],
                    scalar1=GRAD_CLIP, scalar2=-GRAD_CLIP,
                    op0=mybir.AluOpType.min, op1=mybir.AluOpType.max,
                )
                nc.vector.tensor_scalar_mul(m1sh[:], m1sh[:], sc_sb[:, 1:2])
                m1shT = gup.tile([P, EB, P], BF16, tag="m1shT", name="m1shT")
                for dob in range(EB):
                    tp = ps.tile([P, P], BF16, tag="mm", name=f"tp{dob}")
                    nc.tensor.transpose(
                        tp[:], m1sh[:, dob * P : (dob + 1) * P], ident[:]
                    )
                    nc.vector.tensor_copy(out=m1shT[:, dob, :], in_=tp[:])
                wfsh = gup.tile([P, D], BF16, tag="wsh", name="wfsh")
                wfs_sb = gup.tile([P, D], BF16, tag="wfs", name="wfs_sb")
                nc.sync.dma_start(wfs_sb[:], wfs_d[:])
                for dc in range(2):
                    pt = ps.tile([P, 512], F32, tag="mm", name=f"fmm{dc}")
                    for dob in range(EB):
                        nc.tensor.matmul(
                            pt[:],
                            m1shT[:, dob, :],
                            wo[:, dob, dc * 512 : (dc + 1) * 512],
                            start=(dob == 0),
                            stop=(dob == EB - 1),
                        )
                    sl = slice(dc * 512, (dc + 1) * 512)
                    nc.vector.tensor_scalar_mul(wfsh[:, sl], wfs_sb[:, sl], sc_sb[:, 0:1])
                    nc.vector.tensor_tensor(
                        wfsh[:, sl], wfsh[:, sl], pt[:], mybir.AluOpType.add
                    )
                nc.sync.dma_start(ag1_in[:], wfsh[:])
                all_gather(ag1_in, ag1_out)
                wf = wt.tile(WS, BF16, tag="wt", name="WF")
                nc.sync.dma_start(wf[:], ag1_out.rearrange("eb p n -> p eb n"))
                return wf

            def shard_update(ws_d, rs_out, ag_in, ag_out, wname):
                """clip/scale this core's grad shard, decay its base-weight
                shard, AllGather the updated weight, land it in SBUF."""
                gblk = gup.tile([P, D], BF16, tag="gblk", name="gblk")
                nc.sync.dma_start(gblk[:], rs_out[:])
                nc.vector.tensor_scalar(
                    out=gblk[:], in0=gblk[:],
                    scalar1=GRAD_CLIP, scalar2=-GRAD_CLIP,
                    op0=mybir.AluOpType.min, op1=mybir.AluOpType.max,
                )
                nc.vector.tensor_scalar_mul(gblk[:], gblk[:], sc_sb[:, 1:2])
                wsh = gup.tile([P, D], BF16, tag="wsh", name="wsh")
                nc.sync.dma_start(wsh[:], ws_d[:])
                nc.vector.tensor_scalar_mul(wsh[:], wsh[:], sc_sb[:, 0:1])
                nc.vector.tensor_tensor(
                    wsh[:], wsh[:], gblk[:], mybir.AluOpType.add
                )
                nc.sync.dma_start(ag_in[:], wsh[:])
                all_gather(ag_in, ag_out)
                wn = wt.tile(WS, BF16, tag="wt", name=wname)
                nc.sync.dma_start(
                    wn[:], ag_out.rearrange("eb p n -> p eb n")
                )
                return wn

            w0nt = shard_update(w0s_d, rs0_out, ag0_in, ag0_out, "W0NT")
            zT = new_act("bB", "zT")      # after k_tok dead (g0)
            z_post = silu_into(zT)
            wf = None
            for tci in range(NTC):
                if tci == 2:
                    wf = emit_fused_tail()
                qc = qcp.tile([P, EB, 512], BF16, tag="qc", name="qc")
                nc.sync.dma_start(
                    qc[:],
                    bq[:, tci * 512 : (tci + 1) * 512].rearrange(
                        "(eb p) t -> p eb t", p=P
                    ),
                )
                for ob in range(EB):
                    pt = ps.tile([P, 512], F32, tag="mm", name=f"zmm{tci}_{ob}")
                    for e in range(EB):
                        nc.tensor.matmul(
                            pt[:],
                            w0nt[:, e, ob * P : (ob + 1) * P],
                            qc[:, e, :],
                            start=(e == 0),
                            stop=(e == EB - 1),
                        )
                    z_post(pt[:], ob, tci)


            # out token-major: out[tb*128+p, o] = sum_d z^T[d, t-blk] Wf^T[d, o]
            for tb in range(TBL):
                ot = outst.tile([P, D], F32, tag="ot", name="ot")
                for dc in range(2):
                    pt = ps.tile([P, 512], F32, tag="mm", name=f"omm{tb}_{dc}")
                    for db in range(EB):
                        nc.tensor.matmul(
                            pt[:],
                            zT[:, db, tb * P : (tb + 1) * P],
                            wf[:, db, dc * 512 : (dc + 1) * 512],
                            start=(db == 0),
                            stop=(db == EB - 1),
                        )
                    nc.vector.tensor_copy(
                        out=ot[:, dc * 512 : (dc + 1) * 512], in_=pt[:]
                    )
                nc.sync.dma_start(out_d[tb * P : (tb + 1) * P, :], ot[:])

    nc.compile()
    return nc


def _prep(inputs):
    """Host-side shard/layout prep -> list of 8 per-core input maps."""
    bf = ml_dtypes.bfloat16
    x = np.asarray(inputs["x"], np.float32).reshape(B * S, D)

    def t_bf(a):  # transpose + bf16 + contiguous
        return np.ascontiguousarray(np.asarray(a, np.float32).T.astype(bf))

    wg = np.stack(
        [
            np.asarray(inputs["Wg_decay"], np.float32).reshape(D),
            np.asarray(inputs["Wg_lr"], np.float32).reshape(D),
        ]
    )  # [2, D]
    wg_sb = np.ascontiguousarray(wg.reshape(2, EB, P).transpose(2, 0, 1))
    bg = np.array(
        [
            [
                float(np.asarray(inputs["bg_decay"]).reshape(-1)[0]),
                float(np.asarray(inputs["bg_lr"]).reshape(-1)[0]),
            ]
        ],
        np.float32,
    )

    shared = {
        "WKT": t_bf(inputs["Wk"]),
        "WVT": t_bf(inputs["Wv"]),
        "WQT": t_bf(inputs["Wq"]),
        "W0KT": np.ascontiguousarray(
            (
                np.asarray(inputs["Wk"], np.float64).T
                @ np.asarray(inputs["W0"], np.float64).T
            ).astype(np.float32).astype(bf)
        ),  # (W0 @ Wk)^T
        "W1T": t_bf(inputs["W1"]),
        "W1N": np.ascontiguousarray(np.asarray(inputs["W1"], np.float32).astype(bf)),
        "WOT": t_bf(inputs["Wout"]),
        "WG": wg_sb,
        "BG": bg,
    }
    w0t_f = t_bf(inputs["W0"])
    wf_t = np.ascontiguousarray(
        (
            np.asarray(inputs["W1"], np.float64).T
            @ np.asarray(inputs["Wout"], np.float64).T
        ).astype(np.float32).astype(bf)
    )  # (Wout @ W1)^T
    in_maps = []
    for c in range(NC):
        xs = x[c * T : (c + 1) * T]
        m = dict(shared)
        m["xT"] = np.ascontiguousarray(xs.T.astype(bf))
        m["W0TS"] = np.ascontiguousarray(w0t_f[c * P : (c + 1) * P])
        m["WFS"] = np.ascontiguousarray(wf_t[c * P : (c + 1) * P])
        in_maps.append(m)
    return in_maps


def kernel(**inputs) -> np.ndarray:
    if "nc" not in _CACHE:
        _CACHE["nc"] = _build()
    nc = _CACHE["nc"]
    in_maps = _prep(inputs)
    res = run_bass_kernel_spmd(nc, in_maps, core_ids=list(range(NC)))
    out = np.concatenate([res.results[c]["out"] for c in range(NC)], axis=0)
    return out.reshape(B, S, D)

